# revision 4
# baseline (speedup 1.0000x reference)
"""Trainium2 Bass kernel for a dense transformer block (pre-LN, causal MHA + FFN).

Sharding: pure data-parallel over batch — 8 sequences -> 8 NeuronCores, no
collectives. Each core runs the full block on its [2048, 400] slice.

On-chip recipe (per core):
  h   = LN1(x)                       f32 stats, bf16 output
  hT  = transpose(h)  (PE)           [400(c), 2048(t)] bf16, c in 4 chunks of 100
  qT  = bf16(0.1 * Wq[h].T @ hT)     [100(d), 2048] per head   (PSUM f32)
  kT  = bf16(Wk[h].T @ hT)
  v   = bf16(hT.T @ Wv_all)          [2048(t), 400(h*d)] rows
  per (head, 128-row tile): scores = qT.T @ kT (causal-trimmed), +mask on
  diagonal block, exp on ACT with accumulated row sums, probs bf16; attn@V
  accumulates over transposed prob blocks, normalized by 1/rowsum at PSUM
  copy-out; transposed once more into attn_oT [100(d), head, 2048].
  proj = sum_h attn_oT[h].T @ Wo[h]  + residual into x (f32)
  LN2 -> h2T; fc1 in transposed form ffT = relu(W1.T @ h2T + b1) bf16,
  fc2 rows = ffT.T @ W2 + residual + b2 -> out (f32).

All weight reshaping/casting is done host-side in numpy and shipped as extra
ExternalInputs, so the device program has zero setup work.
"""

import numpy as np
import ml_dtypes

import concourse.bass as bass
import concourse.mybir as mybir
import concourse.tile as tile
from concourse import bacc
from concourse.bass_utils import run_bass_kernel_spmd

BF16NP = ml_dtypes.bfloat16
BF16 = mybir.dt.bfloat16
F32 = mybir.dt.float32
AF = mybir.ActivationFunctionType
ALU = mybir.AluOpType
AX = mybir.AxisListType

P = 128          # partitions
B = 8            # batch -> cores
T = 2048         # sequence length
C = 400          # embed dim
H = 4            # heads
D = 100          # head dim
DFF = 1600       # ffn hidden
NT = T // P      # 16 row tiles
NCC = C // D     # 4 contraction chunks of 100
WT = 512         # wide tile for moving operand
NWT = T // WT    # 4
NFC = (DFF + P - 1) // P  # 13 f-chunks (12x128 + 64)
NEG = -1.0e30

LAST_RESULT = None  # BassKernelResults of the most recent run (for test.py)


def _fchunk(fc):
    return min(P, DFF - fc * P)


def build_block():
    nc = bacc.Bacc("TRN2", target_bir_lowering=False, debug=False)

    x_d = nc.dram_tensor("x", [T, C], F32, kind="ExternalInput")
    wq_d = nc.dram_tensor("wqp", [D, H, NCC, D], BF16, kind="ExternalInput")
    wk_d = nc.dram_tensor("wkp", [D, H, NCC, D], BF16, kind="ExternalInput")
    wv_d = nc.dram_tensor("wvp", [D, NCC, C], BF16, kind="ExternalInput")
    wo_d = nc.dram_tensor("wop", [D, H, C], BF16, kind="ExternalInput")
    w1_d = nc.dram_tensor("w1p", [D, NCC, DFF], BF16, kind="ExternalInput")
    w2_d = nc.dram_tensor("w2p", [P, NFC, C], BF16, kind="ExternalInput")
    b1_d = nc.dram_tensor("b1p", [P, NFC], F32, kind="ExternalInput")
    bo_d = nc.dram_tensor("bop", [P, C], F32, kind="ExternalInput")
    b2_d = nc.dram_tensor("b2p", [P, C], F32, kind="ExternalInput")
    g1_d = nc.dram_tensor("g1p", [P, C], F32, kind="ExternalInput")
    be1_d = nc.dram_tensor("be1p", [P, C], F32, kind="ExternalInput")
    g2_d = nc.dram_tensor("g2p", [P, C], F32, kind="ExternalInput")
    be2_d = nc.dram_tensor("be2p", [P, C], F32, kind="ExternalInput")
    mask_d = nc.dram_tensor("maskp", [P, P], F32, kind="ExternalInput")
    id_d = nc.dram_tensor("identp", [P, P], BF16, kind="ExternalInput")
    out_d = nc.dram_tensor("out", [T, C], F32, kind="ExternalOutput")

    with tile.TileContext(nc) as tc:
        with (
            tc.tile_pool(name="consts", bufs=1) as consts,
            tc.tile_pool(name="persist", bufs=1) as persist,
            tc.tile_pool(name="qk", bufs=2) as qk_pool,
            tc.tile_pool(name="pr", bufs=2) as pr_pool,
            tc.tile_pool(name="work", bufs=3) as work,
            tc.tile_pool(name="small", bufs=4) as small,
            tc.tile_pool(name="ps_mm", bufs=3, space="PSUM") as ps_mm,
            tc.tile_pool(name="ps_tr", bufs=3, space="PSUM") as ps_tr,
            tc.tile_pool(name="ps_av", bufs=2, space="PSUM") as ps_av,
        ):
            # ---- constants into SBUF ----
            wq_sb = consts.tile([P, H, NCC, D], BF16, tag="wq")
            nc.sync.dma_start(wq_sb[:D], wq_d[:])
            wk_sb = consts.tile([P, H, NCC, D], BF16, tag="wk")
            nc.sync.dma_start(wk_sb[:D], wk_d[:])
            wv_sb = consts.tile([P, NCC, C], BF16, tag="wv")
            nc.sync.dma_start(wv_sb[:D], wv_d[:])
            wo_sb = consts.tile([P, H, C], BF16, tag="wo")
            nc.sync.dma_start(wo_sb[:D], wo_d[:])
            w1_sb = consts.tile([P, NCC, DFF], BF16, tag="w1")
            nc.sync.dma_start(w1_sb[:D], w1_d[:])
            w2_sb = consts.tile([P, NFC, C], BF16, tag="w2")
            nc.sync.dma_start(w2_sb[:], w2_d[:])
            b1_sb = consts.tile([P, NFC], F32, tag="b1")
            nc.sync.dma_start(b1_sb[:], b1_d[:])
            bo_sb = consts.tile([P, C], F32, tag="bo")
            nc.sync.dma_start(bo_sb[:], bo_d[:])
            b2_sb = consts.tile([P, C], F32, tag="b2")
            nc.sync.dma_start(b2_sb[:], b2_d[:])
            g1_sb = consts.tile([P, C], F32, tag="g1")
            nc.sync.dma_start(g1_sb[:], g1_d[:])
            be1_sb = consts.tile([P, C], F32, tag="be1")
            nc.sync.dma_start(be1_sb[:], be1_d[:])
            g2_sb = consts.tile([P, C], F32, tag="g2")
            nc.sync.dma_start(g2_sb[:], g2_d[:])
            be2_sb = consts.tile([P, C], F32, tag="be2")
            nc.sync.dma_start(be2_sb[:], be2_d[:])
            mask_sb = consts.tile([P, P], F32, tag="mask")
            nc.sync.dma_start(mask_sb[:], mask_d[:])
            id_sb = consts.tile([P, P], BF16, tag="ident")
            nc.sync.dma_start(id_sb[:], id_d[:])
            eps_sb = consts.tile([P, 1], F32, tag="eps")
            nc.vector.memset(eps_sb, 1e-5)

            # ---- x into SBUF, tiled [128, 16, 400] ----
            x_sb = persist.tile([P, NT, C], F32, tag="x")
            nc.sync.dma_start(x_sb[:], x_d.rearrange("(n p) c -> p n c", p=P))

            hT_sb = persist.tile([P, NCC, T], BF16, tag="hT")
            v_sb = persist.tile([P, NT, C], BF16, tag="v")
            ao_sb = persist.tile([P, H, T], BF16, tag="aoT")

            def layernorm_rows(src_ap, g_sb, be_sb, dstT, ti):
                """LN over [128, C] rows of src; bf16 result transposed into
                dstT[:D, cc, ti*P:(ti+1)*P]."""
                stats = small.tile([P, 6], F32, tag="stats")
                nc.vector.bn_stats(out=stats, in_=src_ap)
                mv = small.tile([P, 2], F32, tag="mv")
                nc.vector.bn_aggr(out=mv, in_=stats)
                rstd = small.tile([P, 1], F32, tag="rstd")
                nc.scalar.activation(out=rstd, in_=mv[:, 1:2], func=AF.Sqrt,
                                     bias=eps_sb, scale=1.0)
                nc.vector.reciprocal(out=rstd, in_=rstd)
                hrow = work.tile([P, C], F32, tag="hrow")
                nc.vector.tensor_scalar(out=hrow, in0=src_ap,
                                        scalar1=mv[:, 0:1], scalar2=rstd,
                                        op0=ALU.subtract, op1=ALU.mult)
                nc.vector.tensor_mul(out=hrow, in0=hrow, in1=g_sb)
                hbf = work.tile([P, C], BF16, tag="hbf")
                nc.vector.tensor_add(out=hbf, in0=hrow, in1=be_sb)
                for cc in range(NCC):
                    pt = ps_tr.tile([P, P], BF16, tag="tr")
                    nc.tensor.transpose(pt[:D, :], hbf[:, cc * D:(cc + 1) * D],
                                        id_sb)
                    nc.any.tensor_copy(out=dstT[:D, cc, ti * P:(ti + 1) * P],
                                       in_=pt[:D, :])

            # ---- LN1 + transpose for all row tiles ----
            for ti in range(NT):
                layernorm_rows(x_sb[:, ti, :], g1_sb, be1_sb, hT_sb, ti)

            # ---- V rows for all heads: v = h @ Wv_all ----
            for ti in range(NT):
                psv = ps_mm.tile([P, WT], F32, tag="mm")
                for cc in range(NCC):
                    nc.tensor.matmul(psv[:, :C],
                                     lhsT=hT_sb[:D, cc, ti * P:(ti + 1) * P],
                                     rhs=wv_sb[:D, cc, :],
                                     start=(cc == 0), stop=(cc == NCC - 1))
                nc.any.tensor_copy(out=v_sb[:, ti, :], in_=psv[:, :C])

            # ---- per-head attention ----
            for h in range(H):
                qT = qk_pool.tile([P, T], BF16, tag="qT")
                kT = qk_pool.tile([P, T], BF16, tag="kT")
                for tt in range(NWT):
                    sl = slice(tt * WT, (tt + 1) * WT)
                    psq = ps_mm.tile([P, WT], F32, tag="mm")
                    for cc in range(NCC):
                        nc.tensor.matmul(psq[:D, :], lhsT=wq_sb[:D, h, cc, :],
                                         rhs=hT_sb[:D, cc, sl],
                                         start=(cc == 0), stop=(cc == NCC - 1))
                    nc.scalar.activation(out=qT[:D, sl], in_=psq[:D, :],
                                         func=AF.Copy, scale=0.1)
                    psk = ps_mm.tile([P, WT], F32, tag="mm")
                    for cc in range(NCC):
                        nc.tensor.matmul(psk[:D, :], lhsT=wk_sb[:D, h, cc, :],
                                         rhs=hT_sb[:D, cc, sl],
                                         start=(cc == 0), stop=(cc == NCC - 1))
                    nc.any.tensor_copy(out=kT[:D, sl], in_=psk[:D, :])

                for ti in range(NT):
                    s_hi = (ti + 1) * P
                    nsb = (s_hi + WT - 1) // WT
                    probs = pr_pool.tile([P, T], BF16, tag="probs")
                    sump = small.tile([P, NWT], F32, tag="sump")
                    for sb in range(nsb):
                        ncols = min(WT, s_hi - sb * WT)
                        pss = ps_mm.tile([P, WT], F32, tag="mm")
                        nc.tensor.matmul(pss[:, :ncols],
                                         lhsT=qT[:D, ti * P:(ti + 1) * P],
                                         rhs=kT[:D, sb * WT:sb * WT + ncols],
                                         start=True, stop=True)
                        if sb == nsb - 1:
                            nc.vector.tensor_add(
                                out=pss[:, ncols - P:ncols],
                                in0=pss[:, ncols - P:ncols], in1=mask_sb)
                        nc.scalar.activation(
                            out=probs[:, sb * WT:sb * WT + ncols],
                            in_=pss[:, :ncols], func=AF.Exp,
                            accum_out=sump[:, sb:sb + 1])
                    ssum = small.tile([P, 1], F32, tag="ssum")
                    nc.vector.reduce_sum(ssum, sump[:, :nsb], axis=AX.X)
                    rec = small.tile([P, 1], F32, tag="rec")
                    nc.vector.reciprocal(out=rec, in_=ssum)

                    pso = ps_av.tile([P, P], F32, tag="av")
                    for si in range(ti + 1):
                        ptp = ps_tr.tile([P, P], BF16, tag="tr")
                        nc.tensor.transpose(ptp, probs[:, si * P:(si + 1) * P],
                                            id_sb)
                        aT = work.tile([P, P], BF16, tag="aT")
                        nc.any.tensor_copy(out=aT, in_=ptp)
                        nc.tensor.matmul(pso[:, :D], lhsT=aT,
                                         rhs=v_sb[:, si, h * D:(h + 1) * D],
                                         start=(si == 0), stop=(si == ti))
                    arow = work.tile([P, D], BF16, tag="arow")
                    nc.scalar.activation(out=arow, in_=pso[:, :D],
                                         func=AF.Copy, scale=rec)
                    pta = ps_tr.tile([P, P], BF16, tag="tr")
                    nc.tensor.transpose(pta[:D, :], arow, id_sb)
                    nc.any.tensor_copy(out=ao_sb[:D, h, ti * P:(ti + 1) * P],
                                       in_=pta[:D, :])

            # ---- output projection + residual ----
            for ti in range(NT):
                psp = ps_mm.tile([P, WT], F32, tag="mm")
                for h in range(H):
                    nc.tensor.matmul(psp[:, :C],
                                     lhsT=ao_sb[:D, h, ti * P:(ti + 1) * P],
                                     rhs=wo_sb[:D, h, :],
                                     start=(h == 0), stop=(h == H - 1))
                nc.vector.tensor_add(out=x_sb[:, ti, :], in0=x_sb[:, ti, :],
                                     in1=psp[:, :C])
                nc.vector.tensor_add(out=x_sb[:, ti, :], in0=x_sb[:, ti, :],
                                     in1=bo_sb)

            # ---- FFN in two T-halves ----
            outr = out_d.rearrange("(n p) c -> p n c", p=P)
            HALF = NT // 2  # 8 row tiles, 1024 t-columns per half
            for th in range(2):
                h2T = persist.tile([P, NCC, HALF * P], BF16, tag="hT")
                for tl in range(HALF):
                    layernorm_rows(x_sb[:, th * HALF + tl, :], g2_sb, be2_sb,
                                   h2T, tl)
                ffT = persist.tile([P, NFC, HALF * P], BF16, tag="ffT")
                for fc in range(NFC):
                    fsz = _fchunk(fc)
                    for tt in range(HALF * P // WT):
                        sl = slice(tt * WT, (tt + 1) * WT)
                        psf = ps_mm.tile([P, WT], F32, tag="mm")
                        for cc in range(NCC):
                            nc.tensor.matmul(
                                psf[:fsz, :],
                                lhsT=w1_sb[:D, cc, fc * P:fc * P + fsz],
                                rhs=h2T[:D, cc, sl],
                                start=(cc == 0), stop=(cc == NCC - 1))
                        nc.scalar.activation(out=ffT[:fsz, fc, sl],
                                             in_=psf[:fsz, :], func=AF.Relu,
                                             bias=b1_sb[:fsz, fc:fc + 1],
                                             scale=1.0)
                for tl in range(HALF):
                    ti = th * HALF + tl
                    psg = ps_mm.tile([P, WT], F32, tag="mm")
                    for fc in range(NFC):
                        fsz = _fchunk(fc)
                        nc.tensor.matmul(psg[:, :C],
                                         lhsT=ffT[:fsz, fc,
                                                  tl * P:(tl + 1) * P],
                                         rhs=w2_sb[:fsz, fc, :],
                                         start=(fc == 0), stop=(fc == NFC - 1))
                    orow = work.tile([P, C], F32, tag="orow")
                    nc.vector.tensor_add(out=orow, in0=psg[:, :C],
                                         in1=x_sb[:, ti, :])
                    nc.vector.tensor_add(out=orow, in0=orow, in1=b2_sb)
                    nc.sync.dma_start(outr[:, ti, :], orow)

    nc.finalize()
    return nc


def prep_weights(Wq, Wk, Wv, Wo, bo, W1, b1, W2, b2,
                 ln1_g, ln1_b, ln2_g, ln2_b):
    """Host-side reshape/cast into the layouts the device program expects."""
    f32 = np.float32
    Wq = np.asarray(Wq, f32); Wk = np.asarray(Wk, f32)
    Wv = np.asarray(Wv, f32); Wo = np.asarray(Wo, f32)
    W1 = np.asarray(W1, f32); W2 = np.asarray(W2, f32)
    # [H, C, D] -> [c(100), H, cc, D]
    wqp = Wq.reshape(H, NCC, D, D).transpose(2, 0, 1, 3).astype(BF16NP).copy()
    wkp = Wk.reshape(H, NCC, D, D).transpose(2, 0, 1, 3).astype(BF16NP).copy()
    # [H, C, D] -> [c(100), cc, H*D]
    wvp = (Wv.reshape(H, NCC, D, D).transpose(2, 1, 0, 3)
           .reshape(D, NCC, C).astype(BF16NP).copy())
    # [C, C] -> [c_in_head(100), H, C]
    wop = Wo.reshape(H, D, C).transpose(1, 0, 2).astype(BF16NP).copy()
    # [C, DFF] -> [c(100), cc, DFF]
    w1p = W1.reshape(NCC, D, DFF).transpose(1, 0, 2).astype(BF16NP).copy()
    # [DFF, C] -> [f_in_chunk(128), fc(13), C], zero-padded
    w2p = np.zeros((P, NFC, C), BF16NP)
    b1p = np.zeros((P, NFC), f32)
    for fc in range(NFC):
        fsz = _fchunk(fc)
        w2p[:fsz, fc, :] = W2[fc * P:fc * P + fsz, :].astype(BF16NP)
        b1p[:fsz, fc] = np.asarray(b1, f32)[fc * P:fc * P + fsz]
    tilep = lambda a: np.tile(np.asarray(a, f32).reshape(1, C), (P, 1)).copy()
    mask = np.triu(np.full((P, P), NEG, f32), k=1)
    ident = np.eye(P, dtype=BF16NP)
    return {
        "wqp": wqp, "wkp": wkp, "wvp": wvp, "wop": wop, "w1p": w1p,
        "w2p": w2p, "b1p": b1p, "bop": tilep(bo), "b2p": tilep(b2),
        "g1p": tilep(ln1_g), "be1p": tilep(ln1_b),
        "g2p": tilep(ln2_g), "be2p": tilep(ln2_b),
        "maskp": mask, "identp": ident,
    }


_CACHED_NC = None


def kernel(x, ln1_g, ln1_b, ln2_g, ln2_b, Wq, Wk, Wv, Wo, bo, W1, b1, W2, b2,
           trace=False):
    global _CACHED_NC, LAST_RESULT
    x = np.asarray(x, np.float32)
    assert x.shape == (B, T, C), x.shape
    wmap = prep_weights(Wq, Wk, Wv, Wo, bo, W1, b1, W2, b2,
                        ln1_g, ln1_b, ln2_g, ln2_b)
    if _CACHED_NC is None:
        _CACHED_NC = build_block()
    nc = _CACHED_NC
    in_maps = [dict(wmap, x=np.ascontiguousarray(x[c])) for c in range(B)]
    res = run_bass_kernel_spmd(nc, in_maps, core_ids=list(range(B)),
                               trace=trace)
    LAST_RESULT = res
    out = np.stack([res.results[c]["out"] for c in range(B)])
    return out.astype(np.float32)
